# revision 1
# baseline (speedup 1.0000x reference)
"""Trainium2 Bass kernel for nn_FLAttention (sparse_attention).

Math (per batch b, head h), with q = aq*x+bq, k = ak*x+bk, v = av*x+bv:
  S[i,j] = 1/(|k_j - q_i| + eps);  P = softmax_j(S);  att_i = sum_j P_ij v_j / sqrt(H)
  out = x + sum_h att

Per (b,h) pair and 128-query i-tile (D=1024 -> 8 tiles), engines split so the
DVE (the bottleneck) only runs what no other engine can:
  PE  : d[i,j] = cpe_i + ak*x_j  via one K=2 matmul per 512-chunk -> PSUM
        (lhsT = [cpe_row; ones], rhs = [ones; ak*x]; operand rows built on
        ACT per pair, 4-deep manual double buffering, row-1 writes via SP DMA)
  ACT : a = Abs(d)  PSUM -> SBUF  (PSUM freed immediately; PE runs ahead)
  DVE : amin = min_j a  (reduce on the SBUF abs output)
  ACT/POOL (alternating per tile): nae = -(a + eps)
  DVE : rn = recip_approx_fast(nae) = -1/(|d|+eps)       (custom DVE op)
  ACT+DVE: nm = recip_approx_fast(-(amin+eps)) = -max_j r  (bit-identical to
        the rn value at the argmin, so exp(r - max r) peaks at exactly 0)
  ACT : p = Exp(-rn + nm), accum -> Z = sum_j p
  NSx = sum_j p*x_j * alpha_v/sqrt(H), alternating per tile between
        [POOL p*x then DVE tensor_scalar(*avs)+accum at 2x] and
        [DVE scalar_tensor_tensor fused] to balance engine load.
  POOL: att = NSx * (1/Z from exact DVE reciprocal); head accumulation;
        out = x + sum_h att + sum_h beta_v/sqrt(H); store via SP DMA

Numerical notes: the softmax shift is mathematically exact for any C, so the
approximate reciprocal (51 ULP) is safe as long as the bias uses the same
rounding as the scores - both come from recip_approx_fast, and the row max of
exp is exactly 1. End-to-end error vs the jax reference: ~9e-6 relative.

Sharding: data-parallel over batch: B=16 -> 2 batches per core on 8 cores.
"""
import numpy as np

import concourse.bass as bass
import concourse.bacc as bacc
import concourse.mybir as mybir
import concourse.tile as tile
from concourse.bass_utils import run_bass_kernel_spmd

B, D, H = 16, 1024, 4
N_CORES = 8
BPC = B // N_CORES          # batches per core
NPAIR = BPC * H             # (b,h) pairs per core
NT = D // 128               # i-tiles per pair
EPS = 1e-8
ISH = float(1.0 / np.sqrt(np.float32(H)))  # 1/sqrt(H) = 0.5

F32 = mybir.dt.float32
AX = mybir.AxisListType
OP = mybir.AluOpType
AF = mybir.ActivationFunctionType

EPS_ACT = True   # alternate the eps pass between ACT and POOL


def build_bass():
    nc = bacc.Bacc(
        "TRN2",
        target_bir_lowering=False,
        debug=False,
        enable_asserts=False,
        num_devices=N_CORES,
    )
    x_d = nc.dram_tensor("x", (BPC, D), F32, kind="ExternalInput").ap()
    aq_d = nc.dram_tensor("alpha_q", (1, H), F32, kind="ExternalInput").ap()
    bq_d = nc.dram_tensor("beta_q", (1, H), F32, kind="ExternalInput").ap()
    ak_d = nc.dram_tensor("alpha_k", (1, H), F32, kind="ExternalInput").ap()
    bk_d = nc.dram_tensor("beta_k", (1, H), F32, kind="ExternalInput").ap()
    av_d = nc.dram_tensor("alpha_v", (1, H), F32, kind="ExternalInput").ap()
    bv_d = nc.dram_tensor("beta_v", (1, H), F32, kind="ExternalInput").ap()
    y_d = nc.dram_tensor("y", (BPC, D), F32, kind="ExternalOutput").ap()

    # column-of-128 views: x[b, t*128 + p] <-> view[b, p, t]
    x_col_v = x_d.rearrange("b (t p) -> b p t", p=128)
    y_col_v = y_d.rearrange("b (t p) -> b p t", p=128)

    def bcast_ap(src: bass.AP, n_part: int):
        # replicate a (1, n) DRAM row across n_part partitions (0-stride DMA)
        return bass.AP(
            tensor=src.tensor,
            offset=src.offset,
            ap=[[0, n_part]] + list(src.ap[1:]),
        )

    with tile.TileContext(nc) as tc:
        with (
            tc.tile_pool(name="singles", bufs=1) as singles,
            tc.tile_pool(name="rowp", bufs=3) as rowp,
            tc.tile_pool(name="psum", bufs=3, space="PSUM") as psum,
            tc.tile_pool(name="biga", bufs=5) as biga,       # a tiles
            tc.tile_pool(name="bigae", bufs=5) as bigae,     # a+eps tiles
            tc.tile_pool(name="bigr", bufs=6) as bigr,       # r tiles
            tc.tile_pool(name="bigp", bufs=5) as bigp,       # exp output
            tc.tile_pool(name="bigs", bufs=6) as bigs,       # stt scratch
            tc.tile_pool(name="smalls", bufs=6) as smalls,
        ):
            # ---------- one-time prep ----------
            # params as plain (1,H) tiles on partition 0
            def param_row(src, nm):
                t = singles.tile([1, H], F32, tag=nm)
                nc.gpsimd.dma_start(out=t, in_=src)
                return t

            aqP = param_row(aq_d, "aqP")
            akP = param_row(ak_d, "akP")
            bqP = param_row(bq_d, "bqP")
            bkP = param_row(bk_d, "bkP")

            naqP = singles.tile([1, H], F32, tag="naqP")   # -alpha_q
            nc.vector.tensor_scalar(out=naqP, in0=aqP, scalar1=-1.0, scalar2=None,
                                    op0=OP.mult)
            ccP = singles.tile([1, H], F32, tag="ccP")     # beta_k - beta_q
            nc.vector.tensor_tensor(out=ccP, in0=bkP, in1=bqP, op=OP.subtract)

            # x rows on partition 0, one per batch (matmul operand source)
            xrow = []
            for b in range(BPC):
                xr = singles.tile([1, D], F32, tag=f"xrow{b}")
                nc.gpsimd.dma_start(out=xr, in_=x_d[b:b + 1, :])
                xrow.append(xr)

            ones_row = singles.tile([1, D], F32)
            nc.vector.memset(ones_row, 1.0)
            neps_col = singles.tile([128, 1], F32, tag="neps")
            nc.vector.memset(neps_col, -EPS)

            # K=2 matmul operand tiles, manually double-buffered per pair:
            # lhsT2: p0 = cpe (rewritten per pair), p1 = ones (DMA'd once —
            # engines cannot address base partition 1)
            # rhs2:  p0 = ones (set once), p1 = akx (DMA'd per pair)
            lhsT2 = []
            rhs2 = []
            for k in range(4):
                lt = singles.tile([2, D], F32, tag=f"lhsT2_{k}")
                nc.gpsimd.dma_start(out=lt[1:2, :], in_=ones_row)
                lhsT2.append(lt)
                rt = singles.tile([2, D], F32, tag=f"rhs2_{k}")
                nc.vector.memset(rt[0:1, :], 1.0)
                rhs2.append(rt)

            # value-path params: avs (128,H) = alpha_v/sqrt(H); bvsum (128,1)
            av128 = singles.tile([128, H], F32)
            nc.gpsimd.dma_start(out=av128, in_=bcast_ap(av_d, 128))
            avs = singles.tile([128, H], F32)
            nc.vector.tensor_scalar(out=avs, in0=av128, scalar1=ISH, scalar2=None,
                                    op0=OP.mult)
            bv128 = singles.tile([128, H], F32)
            nc.gpsimd.dma_start(out=bv128, in_=bcast_ap(bv_d, 128))
            bvs = singles.tile([128, H], F32)
            nc.vector.tensor_scalar(out=bvs, in0=bv128, scalar1=ISH, scalar2=None,
                                    op0=OP.mult)
            bvsum = singles.tile([128, 1], F32)
            nc.vector.tensor_reduce(out=bvsum, in_=bvs, axis=AX.X, op=OP.add)

            # x broadcast (128, D) and x column layout (128, NT) per batch
            x_bcast = []
            x_col = []
            for b in range(BPC):
                xb = singles.tile([128, D], F32, tag=f"x_bcast{b}")
                nc.gpsimd.dma_start(
                    out=xb,
                    in_=bass.AP(tensor=x_d.tensor, offset=x_d.offset + b * D,
                                ap=[[0, 128], [1, D]]),
                )
                x_bcast.append(xb)
                xc = singles.tile([128, NT], F32, tag=f"x_col{b}")
                nc.gpsimd.dma_start(out=xc, in_=x_col_v[b])
                x_col.append(xc)

            # ---------- main loops ----------
            for b in range(BPC):
                acc = None
                for h in range(H):
                    p = b * H + h
                    # this pair's matmul operands (K=2): rewrite data rows
                    lt = lhsT2[p % 4]
                    rt = rhs2[p % 4]
                    nc.scalar.activation(
                        out=lt[0:1, :], in_=xrow[b], func=AF.Identity,
                        bias=ccP[0:1, h:h + 1], scale=naqP[0:1, h:h + 1])
                    akx_t = rowp.tile([1, D], F32, tag="akx")
                    nc.scalar.activation(
                        out=akx_t, in_=xrow[b], func=AF.Copy,
                        scale=akP[0:1, h:h + 1])
                    nc.sync.dma_start(out=rt[1:2, :], in_=akx_t)

                    z8 = smalls.tile([128, NT], F32, tag="z8")
                    ns8 = smalls.tile([128, NT], F32, tag="ns8")
                    nm8 = smalls.tile([128, NT], F32, tag="nm8")
                    na8 = smalls.tile([128, NT], F32, tag="na8")
                    amin8 = smalls.tile([128, NT], F32, tag="amin8")
                    for t in range(NT):
                        d2 = psum.tile([128, D], F32, tag="d2")
                        lt_sl = lt[0:2, t * 128:(t + 1) * 128]
                        for c in range(2):
                            js = slice(c * 512, (c + 1) * 512)
                            # dp = cpe_i * 1 + 1 * ak*x_j
                            nc.tensor.matmul(d2[:, c * 512:(c + 1) * 512],
                                             lt_sl, rt[0:2, js],
                                             start=True, stop=True)
                        # a = |d| (ACT table abs is exact), PSUM -> SBUF
                        a_t = biga.tile([128, D], F32, tag="a")
                        nc.scalar.activation(out=a_t, in_=d2, func=AF.Abs)
                        # amin from the SBUF abs output (frees PSUM earlier)
                        nc.vector.tensor_reduce(
                            out=amin8[:, t:t + 1], in_=a_t, axis=AX.X, op=OP.min)
                        # nae = -(a + eps)  (negated so recip gives -r)
                        g = p * NT + t
                        ae_t = bigae.tile([128, D], F32, tag="ae")
                        if g % 2 == 1 and EPS_ACT:
                            nc.scalar.activation(out=ae_t, in_=a_t, func=AF.Identity,
                                                 bias=neps_col, scale=-1.0)
                        else:
                            nc.gpsimd.tensor_scalar(out=ae_t, in0=a_t, scalar1=-1.0,
                                                    scalar2=-EPS, op0=OP.mult, op1=OP.add)
                        # rn = -1/(a+eps)
                        r_t = bigr.tile([128, D], F32, tag="r")
                        nc.vector.reciprocal_approx_fast(out=r_t, in_=ae_t)
                        # nm = recip_fast(-(amin+eps)) (bit-consistent)
                        nc.scalar.activation(
                            out=na8[:, t:t + 1], in_=amin8[:, t:t + 1],
                            func=AF.Identity, bias=neps_col, scale=-1.0)
                        nc.vector.reciprocal_approx_fast(
                            out=nm8[:, t:t + 1], in_=na8[:, t:t + 1])
                        # p = exp(-rn + nm) = exp(r - max r), Z accum
                        p_t = bigp.tile([128, D], F32, tag="p")
                        nc.scalar.activation(out=p_t, in_=r_t, func=AF.Exp,
                                             bias=nm8[:, t:t + 1], scale=-1.0,
                                             accum_out=z8[:, t:t + 1])
                        if g % 2 == 0:
                            # px = p*x on POOL; avs*px + row-sum on DVE at 2x
                            px_t = bigs.tile([128, D], F32, tag="px")
                            nc.gpsimd.tensor_tensor(out=px_t, in0=p_t,
                                                    in1=x_bcast[b], op=OP.mult)
                            s_t = bigs.tile([128, D], F32, tag="s")
                            nc.vector.tensor_scalar(
                                out=s_t, in0=px_t, scalar1=avs[:, h:h + 1],
                                scalar2=0.0, op0=OP.mult, op1=OP.add,
                                accum_out=ns8[:, t:t + 1],
                            )
                        else:
                            # (p * avs) * x fused on DVE
                            s_t = bigs.tile([128, D], F32, tag="s")
                            nc.vector.scalar_tensor_tensor(
                                out=s_t, in0=p_t, scalar=avs[:, h:h + 1],
                                in1=x_bcast[b], op0=OP.mult, op1=OP.mult,
                                accum_out=ns8[:, t:t + 1],
                            )

                    # att_h = avNSx / Z ; acc += att_h
                    rz8 = smalls.tile([128, NT], F32, tag="rz8")
                    nc.vector.reciprocal(out=rz8, in_=z8)
                    acc_new = smalls.tile([128, NT], F32, tag=f"acc{h}")
                    if acc is None:
                        nc.gpsimd.tensor_tensor(out=acc_new, in0=ns8, in1=rz8,
                                                op=OP.mult)
                    else:
                        t2 = smalls.tile([128, NT], F32, tag="t2")
                        nc.gpsimd.tensor_tensor(out=t2, in0=ns8, in1=rz8,
                                                op=OP.mult)
                        nc.gpsimd.tensor_tensor(out=acc_new, in0=acc, in1=t2,
                                                op=OP.add)
                    acc = acc_new

                # y = x + acc + sum_h beta_v/sqrt(H)
                yb8 = smalls.tile([128, NT], F32, tag="yb8")
                nc.gpsimd.tensor_scalar(out=yb8, in0=acc, scalar1=bvsum,
                                        scalar2=None, op0=OP.add)
                y8 = smalls.tile([128, NT], F32, tag="y8")
                nc.gpsimd.tensor_tensor(out=y8, in0=yb8, in1=x_col[b], op=OP.add)
                nc.sync.dma_start(out=y_col_v[b], in_=y8)

    nc.compile()   # bacc passes: split sync waits (1-wait/inst TRN2 limit), etc.
    return nc


_NC_CACHE = {}


def _get_nc():
    if "nc" not in _NC_CACHE:
        _NC_CACHE["nc"] = build_bass()
    return _NC_CACHE["nc"]


def kernel(**inputs) -> np.ndarray:
    x = np.ascontiguousarray(np.asarray(inputs["x"], dtype=np.float32))
    params = {
        k: np.ascontiguousarray(np.asarray(inputs[k], dtype=np.float32))
        for k in ("alpha_q", "beta_q", "alpha_k", "beta_k", "alpha_v", "beta_v")
    }
    nc = _get_nc()
    in_maps = []
    for c in range(N_CORES):
        m = {"x": x[c * BPC:(c + 1) * BPC]}
        m.update(params)
        in_maps.append(m)
    res = run_bass_kernel_spmd(nc, in_maps, core_ids=list(range(N_CORES)))
    return np.concatenate([r["y"] for r in res.results], axis=0)


if __name__ == "__main__":
    rng = np.random.default_rng(0)
    demo = {
        "x": rng.standard_normal((B, D), dtype=np.float32),
        "alpha_q": rng.random((1, H), dtype=np.float32),
        "beta_q": np.zeros((1, H), np.float32),
        "alpha_k": rng.random((1, H), dtype=np.float32),
        "beta_k": np.zeros((1, H), np.float32),
        "alpha_v": rng.random((1, H), dtype=np.float32),
        "beta_v": np.zeros((1, H), np.float32),
    }
    out = kernel(**demo)
    print("kernel output", out.shape, out.dtype)



# revision 6
# speedup vs baseline: 1.3682x; 1.3682x over previous
"""Trainium2 Bass kernel for nn_FLAttention (sparse_attention).

Math (per batch b, head h), with q = aq*x+bq, k = ak*x+bk, v = av*x+bv:
  S[i,j] = 1/(|k_j - q_i| + eps);  P = softmax_j(S);  att_i = sum_j P_ij v_j / sqrt(H)
  out = x + sum_h att

Pipeline per (b,h) pair and 128-query i-tile (D=1024 -> 8 tiles):
  PE  : dp[i,j] = (ak*x_j - aq*x_i + (bk-bq)+eps) via one K=3 matmul per
        512-chunk -> PSUM. Operand rows are built once in the prologue:
        lhsT = [ak*ones; x; ones], rhs = [x; -aq*ones; (bk-bq+eps)*ones],
        where the constant rows come from 0-stride broadcast DMAs (engine-free).
  DVE : custom fused op RECIP_ABSMAX_ANT reads dp from PSUM and computes
        r = recip1NR(max(dp, 2eps-dp)) = ~1/(|d|+eps) (0.4% approx) -> bf16,
        with a free row-max accumulate m (bf16, bit-consistent with r).
  ACT : p = Exp(r - m) -> bf16, accum_out Z (fp32). Row max of p is exactly 1.
  Pool/DVE (split): px = p * x_bcast (bf16 tensor_tensor)
  DVE : tensor_scalar(px * avs_h) at 4x bf16 with accum -> ns column.
  Epilogue per pair: att = ns * (1/Z); accumulate over heads; per batch:
        y = x + sum_h att + sum_h beta_v/sqrt(H).

The custom DVE op is registered at runtime (row 17 of the custom-DVE table);
its 8-stage body is: x=max(Src0, C2-Src0); ~x bit-flip seed; one Newton step.
Approximation error ~0.4% on r only perturbs softmax weight ties between keys
whose values are within the same distance scale - end-to-end rel err ~2e-4.

Sharding: data-parallel over batch: B=16 -> 2 batches per core on 8 cores.
"""
import numpy as np

import concourse.bass as bass
import concourse.bacc as bacc
import concourse.mybir as mybir
import concourse.tile as tile
from concourse.bass_utils import run_bass_kernel_spmd

B, D, H = 16, 1024, 4
N_CORES = 8
BPC = B // N_CORES          # batches per core
NT = D // 128               # i-tiles per pair
EPS = 1e-8
ISH = float(1.0 / np.sqrt(np.float32(H)))  # 1/sqrt(H) = 0.5

F32 = mybir.dt.float32
F32R = mybir.dt.float32r
BF16 = mybir.dt.bfloat16
AX = mybir.AxisListType
OP = mybir.AluOpType
AF = mybir.ActivationFunctionType

F32R_MM = True                      # fp32r matmuls (1 cyc/row vs 4)
DVE_TT_TILES = frozenset({3, 7})    # tiles whose p*x runs on DVE, rest on Pool

# ---------------- custom DVE op: r = ~1/(|d|+eps) with row-max accum --------
from concourse.dve_spec import (Spec, Src0, C0, C1, C2, Zero, Bin, AluOp,
                                 maxx, lower)
from concourse.dve_uop import DveOpSpec
from concourse.dve_ops import DveOp, RECIP_APPROX_FAST_CONSTS
import concourse.dve_ops as dve_ops

RECIP_NAME = "RECIP_ABSMAX_ANT"
C0V = RECIP_APPROX_FAST_CONSTS["s0"]
C1V = RECIP_APPROX_FAST_CONSTS["s1"]


def _recip_absmax_ref(in0, in1, c0, c1, c2):
    # in0 = d+eps; x = max(in0, c2-in0) = |d|+eps (c2 = 2eps);
    # out = 1-NR approx of 1/x; accum = max over free dim, seeded at 0
    x = np.maximum(in0.astype(np.float32),
                   (np.float32(c2) - in0).astype(np.float32))
    not_x = (~x.view(np.int32)).view(np.float32)
    y0 = not_x * np.float32(c0)
    y1 = (y0 * (np.float32(c1) - x * y0)).astype(np.float32)
    P = y1.shape[0]
    body = y1.reshape(P, -1)
    acc = np.maximum(np.float32(0.0), body.max(axis=-1, keepdims=True))
    return body, acc


def _register_recip_op():
    if RECIP_NAME in dve_ops._SUB_OPCODE_FOR_NAME:
        for o in dve_ops.OPS:
            if o.name == RECIP_NAME:
                return o
    x = Bin(AluOp.MAX, Src0, Bin(AluOp.SUBTRACT, C2, Src0))
    nx = Bin(AluOp.BITWISE_NOT, x, x)
    y0 = Bin(AluOp.MULTIPLY, nx, C0)
    t = Bin(AluOp.MULTIPLY, x, y0)
    y1 = Bin(AluOp.MULTIPLY, y0, Bin(AluOp.SUBTRACT, C1, t))
    spec = Spec(body=y1, accum=maxx, accum_init=Zero, reference=_recip_absmax_ref)
    row = max(dve_ops._SUB_OPCODE_FOR_NAME.values()) + 1
    assert row < 0x20
    dve_ops._SUB_OPCODE_FOR_NAME[RECIP_NAME] = row
    shas = {}
    for ver in ("v3", "v4"):
        s = DveOpSpec(name=RECIP_NAME, opcode=row, uops=lower(spec, ver=ver),
                      rd1_en=False)
        shas[ver] = s.sha(ver)
    op = DveOp(RECIP_NAME, spec, subdim=False, uops_sha=shas)
    dve_ops.OPS.append(op)
    dve_ops.CUSTOM_DVE_SPECS[RECIP_NAME] = spec
    return op


RECIP_OP = _register_recip_op()


def build_bass():
    nc = bacc.Bacc(
        "TRN2",
        target_bir_lowering=False,
        debug=False,
        enable_asserts=False,
        num_devices=N_CORES,
    )
    x_d = nc.dram_tensor("x", (BPC, D), F32, kind="ExternalInput").ap()
    aq_d = nc.dram_tensor("alpha_q", (1, H), F32, kind="ExternalInput").ap()
    bq_d = nc.dram_tensor("beta_q", (1, H), F32, kind="ExternalInput").ap()
    ak_d = nc.dram_tensor("alpha_k", (1, H), F32, kind="ExternalInput").ap()
    bk_d = nc.dram_tensor("beta_k", (1, H), F32, kind="ExternalInput").ap()
    av_d = nc.dram_tensor("alpha_v", (1, H), F32, kind="ExternalInput").ap()
    bv_d = nc.dram_tensor("beta_v", (1, H), F32, kind="ExternalInput").ap()
    y_d = nc.dram_tensor("y", (BPC, D), F32, kind="ExternalOutput").ap()

    x_col_v = x_d.rearrange("b (t p) -> b p t", p=128)
    y_col_v = y_d.rearrange("b (t p) -> b p t", p=128)

    MMD = F32R if F32R_MM else F32
    # DRAM scratch for matmul const-row sources: [ak(H); -aq(H); (bk-bq)+eps(H)]
    scr_d = nc.dram_tensor("const_scratch", (1, 3 * H), MMD, kind="Internal").ap()

    def bcast_part(src: bass.AP, n_part: int):
        # replicate a (1, n) row across n_part partitions (0-stride DMA)
        return bass.AP(tensor=src.tensor, offset=src.offset,
                       ap=[[0, n_part]] + list(src.ap[1:]))

    def bcast_free(src_ap: bass.AP, n: int):
        # replicate a single DRAM element along the free dim (0-stride mid dim)
        return bass.AP(tensor=src_ap.tensor, offset=src_ap.offset,
                       ap=[[0, 1], [0, n], [1, 1]])

    with tile.TileContext(nc) as tc:
        with (
            tc.tile_pool(name="singles", bufs=1) as singles,
            tc.tile_pool(name="psum", bufs=3, space="PSUM") as psum,
            tc.tile_pool(name="bigr", bufs=5) as bigr,       # r tiles (bf16)
            tc.tile_pool(name="bigp", bufs=5) as bigp,       # p tiles (bf16)
            tc.tile_pool(name="bigpx", bufs=4) as bigpx,     # px tiles (bf16)
            tc.tile_pool(name="bigs", bufs=3) as bigs,       # ts-val discard
            tc.tile_pool(name="smalls", bufs=6) as smalls,
        ):
            # ---------- one-time prep ----------
            def param_row(src, nm):
                t = singles.tile([1, H], F32, tag=nm)
                nc.sync.dma_start(out=t, in_=src)
                return t

            aqP = param_row(aq_d, "aqP")
            akP = param_row(ak_d, "akP")
            bqP = param_row(bq_d, "bqP")
            bkP = param_row(bk_d, "bkP")

            naqP = singles.tile([1, H], F32, tag="naqP")   # -alpha_q
            nc.vector.tensor_scalar(out=naqP, in0=aqP, scalar1=-1.0, scalar2=None,
                                    op0=OP.mult)
            ccP = singles.tile([1, H], F32, tag="ccP")     # beta_k - beta_q
            nc.vector.tensor_tensor(out=ccP, in0=bkP, in1=bqP, op=OP.subtract)
            cceP = singles.tile([1, H], F32, tag="cceP")   # (bk-bq) + eps
            nc.vector.tensor_scalar(out=cceP, in0=ccP, scalar1=1.0, scalar2=EPS,
                                    op0=OP.mult, op1=OP.add)

            ones_row = singles.tile([1, D], F32, tag="ones_row")
            nc.vector.memset(ones_row, 1.0)

            xrow = []
            for b in range(BPC):
                xr = singles.tile([1, D], F32, tag=f"xrow{b}")
                nc.sync.dma_start(out=xr, in_=x_d[b:b + 1, :])
                xrow.append(xr)

            # const-row sources: [ak; -aq; cce] rounded to MMD, staged in DRAM
            consts3 = singles.tile([1, 3 * H], MMD, tag="consts3")
            nc.vector.tensor_copy(out=consts3[0:1, 0:H], in_=akP)
            nc.vector.tensor_copy(out=consts3[0:1, H:2 * H], in_=naqP)
            nc.vector.tensor_copy(out=consts3[0:1, 2 * H:3 * H], in_=cceP)
            nc.sync.dma_start(out=scr_d, in_=consts3)

            onesR = singles.tile([1, D], MMD, tag="onesR")
            if F32R_MM:
                nc.vector.tensor_copy(out=onesR, in_=ones_row)
                xrowR = []
                for b in range(BPC):
                    xrr = singles.tile([1, D], F32R, tag=f"xrowR{b}")
                    nc.vector.tensor_copy(out=xrr, in_=xrow[b])
                    xrowR.append(xrr)
            else:
                nc.sync.dma_start(out=onesR, in_=ones_row)
                xrowR = xrow

            # persistent matmul operands per pair: lhsT=[ak;x;1], rhs=[x;-aq;cce]
            lhsT3 = []
            rhs3 = []
            for b in range(BPC):
                for h in range(H):
                    lt = singles.tile([3, D], MMD, tag=f"lhsT3_{b}_{h}")
                    nc.sync.dma_start(out=lt[0:1, :],
                                      in_=bcast_free(scr_d[0:1, h:h + 1], D))
                    nc.sync.dma_start(out=lt[1:2, :], in_=xrowR[b])
                    nc.sync.dma_start(out=lt[2:3, :], in_=onesR)
                    lhsT3.append(lt)
                    rt = singles.tile([3, D], MMD, tag=f"rhs3_{b}_{h}")
                    nc.sync.dma_start(out=rt[0:1, :], in_=xrowR[b])
                    nc.sync.dma_start(out=rt[1:2, :],
                                      in_=bcast_free(scr_d[0:1, H + h:H + h + 1], D))
                    nc.sync.dma_start(out=rt[2:3, :],
                                      in_=bcast_free(scr_d[0:1, 2 * H + h:2 * H + h + 1], D))
                    rhs3.append(rt)

            # value-path params: avs (128,H) = alpha_v/sqrt(H); bvsum (128,1)
            av128 = singles.tile([128, H], F32, tag="av128")
            nc.sync.dma_start(out=av128, in_=bcast_part(av_d, 128))
            avs = singles.tile([128, H], F32, tag="avs")
            nc.vector.tensor_scalar(out=avs, in0=av128, scalar1=ISH, scalar2=None,
                                    op0=OP.mult)
            bv128 = singles.tile([128, H], F32, tag="bv128")
            nc.sync.dma_start(out=bv128, in_=bcast_part(bv_d, 128))
            bvs = singles.tile([128, H], F32, tag="bvs")
            nc.vector.tensor_scalar(out=bvs, in0=bv128, scalar1=ISH, scalar2=None,
                                    op0=OP.mult)
            bvsum = singles.tile([128, 1], F32, tag="bvsum")
            nc.vector.tensor_reduce(out=bvsum, in_=bvs, axis=AX.X, op=OP.add)

            # x broadcast (bf16) and x column layout per batch
            xbh = []
            x_col = []
            for b in range(BPC):
                xb = singles.tile([128, D], F32, tag=f"x_bcast{b}")
                nc.sync.dma_start(
                    out=xb,
                    in_=bass.AP(tensor=x_d.tensor, offset=x_d.offset + b * D,
                                ap=[[0, 128], [1, D]]),
                )
                xh = singles.tile([128, D], BF16, tag=f"xbh{b}")
                nc.vector.tensor_copy(out=xh, in_=xb)
                xbh.append(xh)
                xc = singles.tile([128, NT], F32, tag=f"x_col{b}")
                nc.sync.dma_start(out=xc, in_=x_col_v[b])
                x_col.append(xc)

            # ---------- main loops ----------
            for b in range(BPC):
                acc = None
                for h in range(H):
                    p = b * H + h
                    lt = lhsT3[p]
                    rt = rhs3[p]

                    mt8 = smalls.tile([128, NT], BF16, tag="mt8")
                    nm8 = smalls.tile([128, NT], F32, tag="nm8")
                    z8 = smalls.tile([128, NT], F32, tag="z8")
                    ns8 = smalls.tile([128, NT], F32, tag="ns8")

                    # software-pipelined value ops (1 tile behind)
                    pending_val = None

                    def do_val(t, p_t):
                        if t in DVE_TT_TILES:
                            px_t = bigpx.tile([128, D], BF16, tag="px")
                            nc.vector.tensor_tensor(out=px_t, in0=p_t,
                                                    in1=xbh[b], op=OP.mult)
                        else:
                            px_t = bigpx.tile([128, D], BF16, tag="px")
                            nc.gpsimd.tensor_tensor(out=px_t, in0=p_t,
                                                    in1=xbh[b], op=OP.mult)
                        s_t = bigs.tile([128, D], BF16, tag="s")
                        nc.vector.tensor_scalar(
                            out=s_t, in0=px_t, scalar1=avs[:, h:h + 1],
                            scalar2=0.0, op0=OP.mult, op1=OP.add,
                            accum_out=ns8[:, t:t + 1],
                        )

                    for t in range(NT):
                        d2 = psum.tile([128, D], F32, tag="d2")
                        lt_sl = lt[0:3, t * 128:(t + 1) * 128]
                        for c in range(2):
                            js = slice(c * 512, (c + 1) * 512)
                            nc.tensor.matmul(d2[:, js], lt_sl, rt[0:3, js],
                                             start=True, stop=True)
                        # fused: r = ~1/(|d|+eps) bf16, m = row max (bf16)
                        r_t = bigr.tile([128, D], BF16, tag="r")
                        nc.vector._custom_dve(RECIP_OP, out=r_t, in0=d2,
                                              s0=C0V, s1=C1V, imm2=2.0 * EPS,
                                              accum_out=mt8[:, t:t + 1])
                        # bias = -m on ACT (slack engine)
                        nc.scalar.activation(out=nm8[:, t:t + 1],
                                             in_=mt8[:, t:t + 1],
                                             func=AF.Copy, scale=-1.0)
                        # p = exp(r - m) bf16, Z accum fp32
                        p_t = bigp.tile([128, D], BF16, tag="p")
                        nc.scalar.activation(out=p_t, in_=r_t, func=AF.Exp,
                                             bias=nm8[:, t:t + 1], scale=1.0,
                                             accum_out=z8[:, t:t + 1])
                        if pending_val is not None:
                            do_val(*pending_val)
                        pending_val = (t, p_t)
                    do_val(*pending_val)

                    # att_h = ns / Z ; acc += att_h
                    rz8 = smalls.tile([128, NT], F32, tag="rz8")
                    nc.vector.reciprocal(out=rz8, in_=z8)
                    acc_new = smalls.tile([128, NT], F32, tag=f"acc{h}")
                    if acc is None:
                        nc.vector.tensor_tensor(out=acc_new, in0=ns8, in1=rz8,
                                                op=OP.mult)
                    else:
                        t2 = smalls.tile([128, NT], F32, tag="t2")
                        nc.vector.tensor_tensor(out=t2, in0=ns8, in1=rz8,
                                                op=OP.mult)
                        nc.gpsimd.tensor_tensor(out=acc_new, in0=acc, in1=t2,
                                                op=OP.add)
                    acc = acc_new

                # y = x + acc + sum_h beta_v/sqrt(H)
                yb8 = smalls.tile([128, NT], F32, tag="yb8")
                nc.scalar.activation(out=yb8, in_=acc, func=AF.Identity,
                                     bias=bvsum, scale=1.0)
                y8 = smalls.tile([128, NT], F32, tag="y8")
                nc.vector.tensor_tensor(out=y8, in0=yb8, in1=x_col[b], op=OP.add)
                nc.sync.dma_start(out=y_col_v[b], in_=y8)

    nc.compile()
    return nc


_NC_CACHE = {}


def _get_nc():
    if "nc" not in _NC_CACHE:
        _NC_CACHE["nc"] = build_bass()
    return _NC_CACHE["nc"]


def kernel(**inputs) -> np.ndarray:
    x = np.ascontiguousarray(np.asarray(inputs["x"], dtype=np.float32))
    params = {
        k: np.ascontiguousarray(np.asarray(inputs[k], dtype=np.float32))
        for k in ("alpha_q", "beta_q", "alpha_k", "beta_k", "alpha_v", "beta_v")
    }
    nc = _get_nc()
    in_maps = []
    for c in range(N_CORES):
        m = {"x": x[c * BPC:(c + 1) * BPC]}
        m.update(params)
        in_maps.append(m)
    res = run_bass_kernel_spmd(nc, in_maps, core_ids=list(range(N_CORES)))
    return np.concatenate([r["y"] for r in res.results], axis=0)


if __name__ == "__main__":
    rng = np.random.default_rng(0)
    demo = {
        "x": rng.standard_normal((B, D), dtype=np.float32),
        "alpha_q": rng.random((1, H), dtype=np.float32),
        "beta_q": np.zeros((1, H), np.float32),
        "alpha_k": rng.random((1, H), dtype=np.float32),
        "beta_k": np.zeros((1, H), np.float32),
        "alpha_v": rng.random((1, H), dtype=np.float32),
        "beta_v": np.zeros((1, H), np.float32),
    }
    out = kernel(**demo)
    print("kernel output", out.shape, out.dtype)


# revision 12
# speedup vs baseline: 1.5136x; 1.1063x over previous
"""Trainium2 Bass kernel for nn_FLAttention (sparse_attention).

Math (per batch b, head h), with q = aq*x+bq, k = ak*x+bk, v = av*x+bv:
  S[i,j] = 1/(|k_j - q_i| + eps);  P = softmax_j(S);  att_i = sum_j P_ij v_j / sqrt(H)
  out = x + sum_h att

Pipeline per (b,h) pair and 128-query i-tile (D=1024 -> 8 tiles):
  PE  : dp[i,j] = (ak*x_j - aq*x_i + (bk-bq)+eps) via one K=3 matmul per
        512-chunk -> PSUM. Operand rows are built once in the prologue:
        lhsT = [ak*ones; x; ones], rhs = [x; -aq*ones; (bk-bq+eps)*ones],
        where the constant rows come from 0-stride broadcast DMAs (engine-free).
  DVE : custom fused op RECIP_ABSMAX_ANT reads dp from PSUM and computes
        r = recip1NR(max(dp, 2eps-dp)) = ~1/(|d|+eps) (0.4% approx) -> bf16,
        with a free row-max accumulate m (bf16, bit-consistent with r).
  ACT : p = Exp(r - m) -> bf16, accum_out Z (fp32). Row max of p is exactly 1.
  Pool/DVE (split): px = p * x_bcast (bf16 tensor_tensor)
  DVE : tensor_scalar(px * avs_h) at 4x bf16 with accum -> ns column.
  Epilogue per pair: att = ns * (1/Z); accumulate over heads; per batch:
        y = x + sum_h att + sum_h beta_v/sqrt(H).

The custom DVE op is registered at runtime (row 17 of the custom-DVE table);
its 8-stage body is: x=max(Src0, C2-Src0); ~x bit-flip seed; one Newton step.
Approximation error ~0.4% on r only perturbs softmax weight ties between keys
whose values are within the same distance scale - end-to-end rel err ~2e-4.

Sharding: data-parallel over batch: B=16 -> 2 batches per core on 8 cores.
"""
import numpy as np

import concourse.bass as bass
import concourse.bacc as bacc
import concourse.mybir as mybir
import concourse.tile as tile
from concourse.bass_utils import run_bass_kernel_spmd

B, D, H = 16, 1024, 4
N_CORES = 8
BPC = B // N_CORES          # batches per core
NT = D // 128               # i-tiles per pair
EPS = 1e-8
ISH = float(1.0 / np.sqrt(np.float32(H)))  # 1/sqrt(H) = 0.5

F32 = mybir.dt.float32
F32R = mybir.dt.float32r
BF16 = mybir.dt.bfloat16
AX = mybir.AxisListType
OP = mybir.AluOpType
AF = mybir.ActivationFunctionType

F32R_MM = True                      # fp32r matmuls (1 cyc/row vs 4)
DVE_TT_TILES = frozenset({3, 7})    # tiles whose p*x runs on DVE, rest on Pool

# ---------------- custom DVE op: r = ~1/(|d|+eps) with row-max accum --------
from concourse.dve_spec import (Spec, Src0, C0, C1, C2, Zero, Bin, AluOp,
                                 maxx, lower)
from concourse.dve_uop import DveOpSpec
from concourse.dve_ops import DveOp, RECIP_APPROX_FAST_CONSTS
import concourse.dve_ops as dve_ops

RECIP_NAME = "RECIP_ABSMAX_ANT"
C0V = RECIP_APPROX_FAST_CONSTS["s0"]
C1V = RECIP_APPROX_FAST_CONSTS["s1"]


def _recip_absmax_ref(in0, in1, c0, c1, c2):
    # in0 = d+eps; x = max(in0, c2-in0) = |d|+eps (c2 = 2eps);
    # out = 1-NR approx of 1/x; accum = max over free dim, seeded at 0
    x = np.maximum(in0.astype(np.float32),
                   (np.float32(c2) - in0).astype(np.float32))
    not_x = (~x.view(np.int32)).view(np.float32)
    y0 = not_x * np.float32(c0)
    y1 = (y0 * (np.float32(c1) - x * y0)).astype(np.float32)
    P = y1.shape[0]
    body = y1.reshape(P, -1)
    acc = np.maximum(np.float32(0.0), body.max(axis=-1, keepdims=True))
    return body, acc


def _register_recip_op():
    if RECIP_NAME in dve_ops._SUB_OPCODE_FOR_NAME:
        for o in dve_ops.OPS:
            if o.name == RECIP_NAME:
                return o
    x = Bin(AluOp.MAX, Src0, Bin(AluOp.SUBTRACT, C2, Src0))
    nx = Bin(AluOp.BITWISE_NOT, x, x)
    y0 = Bin(AluOp.MULTIPLY, nx, C0)
    t = Bin(AluOp.MULTIPLY, x, y0)
    y1 = Bin(AluOp.MULTIPLY, y0, Bin(AluOp.SUBTRACT, C1, t))
    spec = Spec(body=y1, accum=maxx, accum_init=Zero, reference=_recip_absmax_ref)
    row = max(dve_ops._SUB_OPCODE_FOR_NAME.values()) + 1
    assert row < 0x20
    dve_ops._SUB_OPCODE_FOR_NAME[RECIP_NAME] = row
    shas = {}
    for ver in ("v3", "v4"):
        s = DveOpSpec(name=RECIP_NAME, opcode=row, uops=lower(spec, ver=ver),
                      rd1_en=False)
        shas[ver] = s.sha(ver)
    op = DveOp(RECIP_NAME, spec, subdim=False, uops_sha=shas)
    dve_ops.OPS.append(op)
    dve_ops.CUSTOM_DVE_SPECS[RECIP_NAME] = spec
    return op


RECIP_OP = _register_recip_op()


def build_bass():
    nc = bacc.Bacc(
        "TRN2",
        target_bir_lowering=False,
        debug=False,
        enable_asserts=False,
        num_devices=N_CORES,
    )
    x_d = nc.dram_tensor("x", (BPC, D), F32, kind="ExternalInput").ap()
    aq_d = nc.dram_tensor("alpha_q", (1, H), F32, kind="ExternalInput").ap()
    bq_d = nc.dram_tensor("beta_q", (1, H), F32, kind="ExternalInput").ap()
    ak_d = nc.dram_tensor("alpha_k", (1, H), F32, kind="ExternalInput").ap()
    bk_d = nc.dram_tensor("beta_k", (1, H), F32, kind="ExternalInput").ap()
    av_d = nc.dram_tensor("alpha_v", (1, H), F32, kind="ExternalInput").ap()
    bv_d = nc.dram_tensor("beta_v", (1, H), F32, kind="ExternalInput").ap()
    y_d = nc.dram_tensor("y", (BPC, D), F32, kind="ExternalOutput").ap()

    x_col_v = x_d.rearrange("b (t p) -> b p t", p=128)
    y_col_v = y_d.rearrange("b (t p) -> b p t", p=128)

    MMD = F32R if F32R_MM else F32
    # DRAM scratch for matmul const-row sources: [ak(H); -aq(H); (bk-bq)+eps(H)]
    scr_d = nc.dram_tensor("const_scratch", (1, 3 * H), MMD, kind="Internal").ap()

    def bcast_part(src: bass.AP, n_part: int):
        # replicate a (1, n) row across n_part partitions (0-stride DMA)
        return bass.AP(tensor=src.tensor, offset=src.offset,
                       ap=[[0, n_part]] + list(src.ap[1:]))

    def bcast_free(src_ap: bass.AP, n: int):
        # replicate a single DRAM element along the free dim (0-stride mid dim)
        return bass.AP(tensor=src_ap.tensor, offset=src_ap.offset,
                       ap=[[0, 1], [0, n], [1, 1]])

    with tile.TileContext(nc) as tc:
        with (
            tc.tile_pool(name="singles", bufs=1) as singles,
            tc.tile_pool(name="psum", bufs=3, space="PSUM") as psum,
            tc.tile_pool(name="bigr", bufs=5) as bigr,       # r tiles (bf16)
            tc.tile_pool(name="bigp", bufs=5) as bigp,       # p tiles (bf16)
            tc.tile_pool(name="bigpx", bufs=4) as bigpx,     # px tiles (bf16)
            tc.tile_pool(name="bigs", bufs=3) as bigs,       # ts-val discard
            tc.tile_pool(name="smalls", bufs=6) as smalls,
        ):
            # ---------- one-time prep ----------
            # x assets for batch 0 first (needed by the first value pass);
            # issued from the ACT queue so SP can stream operand DMAs in parallel
            x_bcast = []
            xbh = []
            x_col = []
            xrow = []
            for b in range(BPC):
                xb = singles.tile([128, D], F32, tag=f"x_bcast{b}")
                nc.scalar.dma_start(
                    out=xb,
                    in_=bass.AP(tensor=x_d.tensor, offset=x_d.offset + b * D,
                                ap=[[0, 128], [1, D]]),
                )
                x_bcast.append(xb)
                xh = singles.tile([128, D], BF16, tag=f"xbh{b}")
                nc.vector.tensor_copy(out=xh, in_=xb)
                xbh.append(xh)
                xc = singles.tile([128, NT], F32, tag=f"x_col{b}")
                nc.scalar.dma_start(out=xc, in_=x_col_v[b])
                x_col.append(xc)
                xr = singles.tile([1, D], F32, tag=f"xrow{b}")
                nc.scalar.dma_start(out=xr, in_=x_d[b:b + 1, :])
                xrow.append(xr)

            def param_row(src, nm):
                t = singles.tile([1, H], F32, tag=nm)
                nc.sync.dma_start(out=t, in_=src)
                return t

            aqP = param_row(aq_d, "aqP")
            akP = param_row(ak_d, "akP")
            bqP = param_row(bq_d, "bqP")
            bkP = param_row(bk_d, "bkP")

            naqP = singles.tile([1, H], F32, tag="naqP")   # -alpha_q
            nc.vector.tensor_scalar(out=naqP, in0=aqP, scalar1=-1.0, scalar2=None,
                                    op0=OP.mult)
            ccP = singles.tile([1, H], F32, tag="ccP")     # beta_k - beta_q
            nc.vector.tensor_tensor(out=ccP, in0=bkP, in1=bqP, op=OP.subtract)
            cceP = singles.tile([1, H], F32, tag="cceP")   # (bk-bq) + eps
            nc.vector.tensor_scalar(out=cceP, in0=ccP, scalar1=1.0, scalar2=EPS,
                                    op0=OP.mult, op1=OP.add)

            ones_row = singles.tile([1, D], F32, tag="ones_row")
            nc.gpsimd.memset(ones_row, 1.0)

            # const-row sources: [ak; -aq; cce] rounded to MMD, staged in DRAM
            consts3 = singles.tile([1, 3 * H], MMD, tag="consts3")
            nc.vector.tensor_copy(out=consts3[0:1, 0:H], in_=akP)
            nc.vector.tensor_copy(out=consts3[0:1, H:2 * H], in_=naqP)
            nc.vector.tensor_copy(out=consts3[0:1, 2 * H:3 * H], in_=cceP)
            nc.sync.dma_start(out=scr_d, in_=consts3)

            onesR = singles.tile([1, D], MMD, tag="onesR")
            if F32R_MM:
                nc.gpsimd.tensor_copy(out=onesR, in_=ones_row)
                xrowR = []
                for b in range(BPC):
                    xrr = singles.tile([1, D], F32R, tag=f"xrowR{b}")
                    nc.gpsimd.tensor_copy(out=xrr, in_=xrow[b])
                    xrowR.append(xrr)
            else:
                nc.sync.dma_start(out=onesR, in_=ones_row)
                xrowR = xrow

            # value-path params: avs (128,H) = alpha_v/sqrt(H); bvsum (128,1)
            av128 = singles.tile([128, H], F32, tag="av128")
            nc.sync.dma_start(out=av128, in_=bcast_part(av_d, 128))
            avs = singles.tile([128, H], F32, tag="avs")
            nc.vector.tensor_scalar(out=avs, in0=av128, scalar1=ISH, scalar2=None,
                                    op0=OP.mult)
            bv128 = singles.tile([128, H], F32, tag="bv128")
            nc.sync.dma_start(out=bv128, in_=bcast_part(bv_d, 128))
            bvs = singles.tile([128, H], F32, tag="bvs")
            nc.vector.tensor_scalar(out=bvs, in0=bv128, scalar1=ISH, scalar2=None,
                                    op0=OP.mult)
            bvsum = singles.tile([128, 1], F32, tag="bvsum")
            nc.vector.tensor_reduce(out=bvsum, in_=bvs, axis=AX.X, op=OP.add)

            # persistent matmul operands per pair: lhsT=[ak;x;1], rhs=[x;-aq;cce]
            lhsT3 = []
            rhs3 = []
            for b in range(BPC):
                for h in range(H):
                    lt = singles.tile([3, D], MMD, tag=f"lhsT3_{b}_{h}")
                    nc.sync.dma_start(out=lt[0:1, :],
                                      in_=bcast_free(scr_d[0:1, h:h + 1], D))
                    nc.sync.dma_start(out=lt[1:2, :], in_=xrowR[b])
                    nc.sync.dma_start(out=lt[2:3, :], in_=onesR)
                    lhsT3.append(lt)
                    rt = singles.tile([3, D], MMD, tag=f"rhs3_{b}_{h}")
                    nc.sync.dma_start(out=rt[0:1, :], in_=xrowR[b])
                    nc.sync.dma_start(out=rt[1:2, :],
                                      in_=bcast_free(scr_d[0:1, H + h:H + h + 1], D))
                    nc.sync.dma_start(out=rt[2:3, :],
                                      in_=bcast_free(scr_d[0:1, 2 * H + h:2 * H + h + 1], D))
                    rhs3.append(rt)

            # ---------- main loops ----------
            for b in range(BPC):
                acc = None
                for h in range(H):
                    p = b * H + h
                    lt = lhsT3[p]
                    rt = rhs3[p]

                    mt8 = smalls.tile([128, NT], F32, tag="mt8")
                    nm8 = smalls.tile([128, NT], F32, tag="nm8")
                    z8 = smalls.tile([128, NT], F32, tag="z8")
                    ns8 = smalls.tile([128, NT], F32, tag="ns8")

                    # software-pipelined value ops (2 tiles behind so the DVE
                    # never stalls on Pool's px)
                    pending_val = []

                    def do_val(t, p_t):
                        px_t = bigpx.tile([128, D], BF16, tag="px")
                        if t in DVE_TT_TILES:
                            nc.vector.tensor_tensor(out=px_t, in0=p_t,
                                                    in1=xbh[b], op=OP.mult)
                        else:
                            nc.gpsimd.tensor_tensor(out=px_t, in0=p_t,
                                                    in1=x_bcast[b], op=OP.mult)
                        s_t = bigs.tile([128, D], BF16, tag="s")
                        nc.vector.tensor_scalar(
                            out=s_t, in0=px_t, scalar1=avs[:, h:h + 1],
                            scalar2=0.0, op0=OP.mult, op1=OP.add,
                            accum_out=ns8[:, t:t + 1],
                        )

                    for t in range(NT):
                        d2 = psum.tile([128, D], F32, tag="d2")
                        lt_sl = lt[0:3, t * 128:(t + 1) * 128]
                        for c in range(2):
                            js = slice(c * 512, (c + 1) * 512)
                            nc.tensor.matmul(d2[:, js], lt_sl, rt[0:3, js],
                                             start=True, stop=True)
                        # fused: r = ~1/(|d|+eps) fp32, m = row max (fp32)
                        r_t = bigr.tile([128, D], F32, tag="r")
                        nc.vector._custom_dve(RECIP_OP, out=r_t, in0=d2,
                                              s0=C0V, s1=C1V, imm2=2.0 * EPS,
                                              accum_out=mt8[:, t:t + 1])
                        # bias = -m on ACT (slack engine)
                        nc.scalar.activation(out=nm8[:, t:t + 1],
                                             in_=mt8[:, t:t + 1],
                                             func=AF.Copy, scale=-1.0)
                        # p = exp(r - m), Z accum fp32; p dtype matches the
                        # tile's value-path engine (bf16 only where DVE's
                        # 2x tensor_tensor needs 2-byte operands)
                        p_t = bigp.tile([128, D],
                                        BF16 if t in DVE_TT_TILES else F32,
                                        tag="p")
                        nc.scalar.activation(out=p_t, in_=r_t, func=AF.Exp,
                                             bias=nm8[:, t:t + 1], scale=1.0,
                                             accum_out=z8[:, t:t + 1])
                        pending_val.append((t, p_t))
                        if len(pending_val) > 2:
                            do_val(*pending_val.pop(0))
                    for pv in pending_val:
                        do_val(*pv)

                    # att_h = ns / Z ; acc += att_h
                    rz8 = smalls.tile([128, NT], F32, tag="rz8")
                    nc.vector.reciprocal(out=rz8, in_=z8)
                    acc_new = smalls.tile([128, NT], F32, tag=f"acc{h}")
                    if acc is None:
                        nc.vector.tensor_tensor(out=acc_new, in0=ns8, in1=rz8,
                                                op=OP.mult)
                    else:
                        t2 = smalls.tile([128, NT], F32, tag="t2")
                        nc.vector.tensor_tensor(out=t2, in0=ns8, in1=rz8,
                                                op=OP.mult)
                        nc.gpsimd.tensor_tensor(out=acc_new, in0=acc, in1=t2,
                                                op=OP.add)
                    acc = acc_new

                # y = x + acc + sum_h beta_v/sqrt(H)
                yb8 = smalls.tile([128, NT], F32, tag="yb8")
                nc.scalar.activation(out=yb8, in_=acc, func=AF.Identity,
                                     bias=bvsum, scale=1.0)
                y8 = smalls.tile([128, NT], F32, tag="y8")
                nc.vector.tensor_tensor(out=y8, in0=yb8, in1=x_col[b], op=OP.add)
                nc.sync.dma_start(out=y_col_v[b], in_=y8)

    nc.compile()
    return nc


_NC_CACHE = {}


def _get_nc():
    if "nc" not in _NC_CACHE:
        _NC_CACHE["nc"] = build_bass()
    return _NC_CACHE["nc"]


def kernel(**inputs) -> np.ndarray:
    x = np.ascontiguousarray(np.asarray(inputs["x"], dtype=np.float32))
    params = {
        k: np.ascontiguousarray(np.asarray(inputs[k], dtype=np.float32))
        for k in ("alpha_q", "beta_q", "alpha_k", "beta_k", "alpha_v", "beta_v")
    }
    nc = _get_nc()
    in_maps = []
    for c in range(N_CORES):
        m = {"x": x[c * BPC:(c + 1) * BPC]}
        m.update(params)
        in_maps.append(m)
    res = run_bass_kernel_spmd(nc, in_maps, core_ids=list(range(N_CORES)))
    return np.concatenate([r["y"] for r in res.results], axis=0)


if __name__ == "__main__":
    rng = np.random.default_rng(0)
    demo = {
        "x": rng.standard_normal((B, D), dtype=np.float32),
        "alpha_q": rng.random((1, H), dtype=np.float32),
        "beta_q": np.zeros((1, H), np.float32),
        "alpha_k": rng.random((1, H), dtype=np.float32),
        "beta_k": np.zeros((1, H), np.float32),
        "alpha_v": rng.random((1, H), dtype=np.float32),
        "beta_v": np.zeros((1, H), np.float32),
    }
    out = kernel(**demo)
    print("kernel output", out.shape, out.dtype)


# revision 19
# speedup vs baseline: 1.6168x; 1.0682x over previous
"""Trainium2 Bass kernel for nn_FLAttention (sparse_attention).

Math (per batch b, head h), with q = aq*x+bq, k = ak*x+bk, v = av*x+bv:
  S[i,j] = 1/(|k_j - q_i| + eps);  P = softmax_j(S);  att_i = sum_j P_ij v_j / sqrt(H)
  out = x + sum_h att

Pipeline per (b,h) pair and 128-query i-tile (D=1024 -> 8 tiles):
  PE  : dp[i,j] = (ak*x_j - aq*x_i + (bk-bq)+eps) via one K=3 matmul per
        512-chunk -> PSUM. Operand rows are built once in the prologue:
        lhsT = [ak*ones; x; ones], rhs = [x; -aq*ones; (bk-bq+eps)*ones],
        where the constant rows come from 0-stride broadcast DMAs (engine-free).
  DVE : custom fused op RECIP_ABSMAX_ANT reads dp from PSUM and computes
        r = recip1NR(max(dp, 2eps-dp)) = ~1/(|d|+eps) (0.4% approx) -> bf16,
        with a free row-max accumulate m (bf16, bit-consistent with r).
  ACT : p = Exp(r - m) -> bf16, accum_out Z (fp32). Row max of p is exactly 1.
  Pool/DVE (split): px = p * x_bcast (bf16 tensor_tensor)
  DVE : tensor_scalar(px * avs_h) at 4x bf16 with accum -> ns column.
  Epilogue per pair: att = ns * (1/Z); accumulate over heads; per batch:
        y = x + sum_h att + sum_h beta_v/sqrt(H).

The custom DVE op is registered at runtime (row 17 of the custom-DVE table);
its 8-stage body is: x=max(Src0, C2-Src0); ~x bit-flip seed; one Newton step.
Approximation error ~0.4% on r only perturbs softmax weight ties between keys
whose values are within the same distance scale - end-to-end rel err ~2e-4.

Sharding: data-parallel over batch: B=16 -> 2 batches per core on 8 cores.
"""
import numpy as np

import concourse.bass as bass
import concourse.bacc as bacc
import concourse.mybir as mybir
import concourse.tile as tile
from concourse.bass_utils import run_bass_kernel_spmd

B, D, H = 16, 1024, 4
N_CORES = 8
BPC = B // N_CORES          # batches per core
NPAIR = BPC * H             # (b,h) pairs per core
NT = D // 128               # i-tiles per pair
EPS = 1e-8
ISH = float(1.0 / np.sqrt(np.float32(H)))  # 1/sqrt(H) = 0.5

F32 = mybir.dt.float32
F32R = mybir.dt.float32r
BF16 = mybir.dt.bfloat16
AX = mybir.AxisListType
OP = mybir.AluOpType
AF = mybir.ActivationFunctionType

F32R_MM = True                      # fp32r matmuls (1 cyc/row vs 4)
DVE_TT_TILES = frozenset({3, 7})    # tiles whose p*x runs on DVE, rest on Pool

# ---------------- custom DVE op: r = ~1/(|d|+eps) with row-max accum --------
from concourse.dve_spec import (Spec, Src0, C0, C1, C2, Zero, Bin, AluOp,
                                 maxx, lower)
from concourse.dve_uop import DveOpSpec
from concourse.dve_ops import DveOp, RECIP_APPROX_FAST_CONSTS
import concourse.dve_ops as dve_ops

RECIP_NAME = "RECIP_ABSMAX_ANT"
C0V = RECIP_APPROX_FAST_CONSTS["s0"]
C1V = RECIP_APPROX_FAST_CONSTS["s1"]


def _recip_absmax_ref(in0, in1, c0, c1, c2):
    # in0 = d+eps; x = max(in0, c2-in0) = |d|+eps (c2 = 2eps);
    # out = 1-NR approx of 1/x; accum = max over free dim, seeded at 0
    x = np.maximum(in0.astype(np.float32),
                   (np.float32(c2) - in0).astype(np.float32))
    not_x = (~x.view(np.int32)).view(np.float32)
    y0 = not_x * np.float32(c0)
    y1 = (y0 * (np.float32(c1) - x * y0)).astype(np.float32)
    P = y1.shape[0]
    body = y1.reshape(P, -1)
    acc = np.maximum(np.float32(0.0), body.max(axis=-1, keepdims=True))
    return body, acc


def _register_recip_op():
    if RECIP_NAME in dve_ops._SUB_OPCODE_FOR_NAME:
        for o in dve_ops.OPS:
            if o.name == RECIP_NAME:
                return o
    x = Bin(AluOp.MAX, Src0, Bin(AluOp.SUBTRACT, C2, Src0))
    nx = Bin(AluOp.BITWISE_NOT, x, x)
    y0 = Bin(AluOp.MULTIPLY, nx, C0)
    t = Bin(AluOp.MULTIPLY, x, y0)
    y1 = Bin(AluOp.MULTIPLY, y0, Bin(AluOp.SUBTRACT, C1, t))
    spec = Spec(body=y1, accum=maxx, accum_init=Zero, reference=_recip_absmax_ref)
    row = max(dve_ops._SUB_OPCODE_FOR_NAME.values()) + 1
    assert row < 0x20
    dve_ops._SUB_OPCODE_FOR_NAME[RECIP_NAME] = row
    shas = {}
    for ver in ("v3", "v4"):
        s = DveOpSpec(name=RECIP_NAME, opcode=row, uops=lower(spec, ver=ver),
                      rd1_en=False)
        shas[ver] = s.sha(ver)
    op = DveOp(RECIP_NAME, spec, subdim=False, uops_sha=shas)
    dve_ops.OPS.append(op)
    dve_ops.CUSTOM_DVE_SPECS[RECIP_NAME] = spec
    return op


RECIP_OP = _register_recip_op()


def build_bass():
    nc = bacc.Bacc(
        "TRN2",
        target_bir_lowering=False,
        debug=False,
        enable_asserts=False,
        num_devices=N_CORES,
    )
    x_d = nc.dram_tensor("x", (BPC, D), F32, kind="ExternalInput").ap()
    aq_d = nc.dram_tensor("alpha_q", (1, H), F32, kind="ExternalInput").ap()
    bq_d = nc.dram_tensor("beta_q", (1, H), F32, kind="ExternalInput").ap()
    ak_d = nc.dram_tensor("alpha_k", (1, H), F32, kind="ExternalInput").ap()
    bk_d = nc.dram_tensor("beta_k", (1, H), F32, kind="ExternalInput").ap()
    av_d = nc.dram_tensor("alpha_v", (1, H), F32, kind="ExternalInput").ap()
    bv_d = nc.dram_tensor("beta_v", (1, H), F32, kind="ExternalInput").ap()
    y_d = nc.dram_tensor("y", (BPC, D), F32, kind="ExternalOutput").ap()

    x_col_v = x_d.rearrange("b (t p) -> b p t", p=128)
    y_col_v = y_d.rearrange("b (t p) -> b p t", p=128)

    MMD = F32R if F32R_MM else F32
    # DRAM scratch for matmul const-row sources: [ak(H); -aq(H); (bk-bq)+eps(H)]
    scr_d = nc.dram_tensor("const_scratch", (1, 3 * H), MMD, kind="Internal").ap()

    def bcast_part(src: bass.AP, n_part: int):
        # replicate a (1, n) row across n_part partitions (0-stride DMA)
        return bass.AP(tensor=src.tensor, offset=src.offset,
                       ap=[[0, n_part]] + list(src.ap[1:]))

    def bcast_free(src_ap: bass.AP, n: int):
        # replicate a single DRAM element along the free dim (0-stride mid dim)
        return bass.AP(tensor=src_ap.tensor, offset=src_ap.offset,
                       ap=[[0, 1], [0, n], [1, 1]])

    with tile.TileContext(nc) as tc:
        with (
            tc.tile_pool(name="singles", bufs=1) as singles,
            tc.tile_pool(name="psum", bufs=3, space="PSUM") as psum,
            tc.tile_pool(name="bigr", bufs=5) as bigr,       # r tiles (bf16)
            tc.tile_pool(name="bigp", bufs=5) as bigp,       # p tiles (bf16)
            tc.tile_pool(name="bigpx", bufs=4) as bigpx,     # px tiles (bf16)
            tc.tile_pool(name="bigs", bufs=3) as bigs,       # ts-val discard
            tc.tile_pool(name="smalls", bufs=6) as smalls,
        ):
            # ---------- one-time prep ----------
            # params first on the SP queue (head of the const-row chain)
            def param_row(src, nm):
                t = singles.tile([1, H], F32, tag=nm)
                nc.sync.dma_start(out=t, in_=src)
                return t

            aqP = param_row(aq_d, "aqP")
            akP = param_row(ak_d, "akP")
            bqP = param_row(bq_d, "bqP")
            bkP = param_row(bk_d, "bkP")

            # x assets on the ACT queue (parallel with SP's stream)
            x_bcast = []
            xbh = []
            x_col = []
            xrow = []
            for b in range(BPC):
                xb = singles.tile([128, D], F32, tag=f"x_bcast{b}")
                nc.scalar.dma_start(
                    out=xb,
                    in_=bass.AP(tensor=x_d.tensor, offset=x_d.offset + b * D,
                                ap=[[0, 128], [1, D]]),
                )
                x_bcast.append(xb)
                xh = singles.tile([128, D], BF16, tag=f"xbh{b}")
                nc.gpsimd.tensor_copy(out=xh, in_=xb)
                xbh.append(xh)
                xc = singles.tile([128, NT], F32, tag=f"x_col{b}")
                nc.scalar.dma_start(out=xc, in_=x_col_v[b])
                x_col.append(xc)
                xr = singles.tile([1, D], F32, tag=f"xrow{b}")
                nc.scalar.dma_start(out=xr, in_=x_d[b:b + 1, :])
                xrow.append(xr)

            naqP = singles.tile([1, H], F32, tag="naqP")   # -alpha_q
            nc.vector.tensor_scalar(out=naqP, in0=aqP, scalar1=-1.0, scalar2=None,
                                    op0=OP.mult)
            ccP = singles.tile([1, H], F32, tag="ccP")     # beta_k - beta_q
            nc.vector.tensor_tensor(out=ccP, in0=bkP, in1=bqP, op=OP.subtract)
            cceP = singles.tile([1, H], F32, tag="cceP")   # (bk-bq) + eps
            nc.vector.tensor_scalar(out=cceP, in0=ccP, scalar1=1.0, scalar2=EPS,
                                    op0=OP.mult, op1=OP.add)

            ones_row = singles.tile([1, D], F32, tag="ones_row")
            nc.gpsimd.memset(ones_row, 1.0)

            # const-row sources: [ak; -aq; cce] rounded to MMD, staged in DRAM
            consts3 = singles.tile([1, 3 * H], MMD, tag="consts3")
            nc.vector.tensor_copy(out=consts3[0:1, 0:H], in_=akP)
            nc.vector.tensor_copy(out=consts3[0:1, H:2 * H], in_=naqP)
            nc.vector.tensor_copy(out=consts3[0:1, 2 * H:3 * H], in_=cceP)
            nc.sync.dma_start(out=scr_d, in_=consts3)

            onesR = singles.tile([1, D], MMD, tag="onesR")
            if F32R_MM:
                nc.gpsimd.tensor_copy(out=onesR, in_=ones_row)
                xrowR = []
                for b in range(BPC):
                    xrr = singles.tile([1, D], F32R, tag=f"xrowR{b}")
                    nc.gpsimd.tensor_copy(out=xrr, in_=xrow[b])
                    xrowR.append(xrr)
            else:
                nc.sync.dma_start(out=onesR, in_=ones_row)
                xrowR = xrow

            # value-path params on the ACT queue
            av128 = singles.tile([128, H], F32, tag="av128")
            nc.scalar.dma_start(out=av128, in_=bcast_part(av_d, 128))
            avs = singles.tile([128, H], F32, tag="avs")
            nc.vector.tensor_scalar(out=avs, in0=av128, scalar1=ISH, scalar2=None,
                                    op0=OP.mult)
            bv128 = singles.tile([128, H], F32, tag="bv128")
            nc.scalar.dma_start(out=bv128, in_=bcast_part(bv_d, 128))
            bvs = singles.tile([128, H], F32, tag="bvs")
            nc.vector.tensor_scalar(out=bvs, in0=bv128, scalar1=ISH, scalar2=None,
                                    op0=OP.mult)
            bvsum = singles.tile([128, 1], F32, tag="bvsum")
            nc.vector.tensor_reduce(out=bvsum, in_=bvs, axis=AX.X, op=OP.add)

            # persistent matmul operands, one big tile per side; pair p's
            # operand block is columns [p*D, (p+1)*D):
            #   lhsT rows = [ak; x; 1], rhs rows = [x; -aq; cce]
            big_lhsT = singles.tile([3, NPAIR * D], MMD, tag="big_lhsT")
            big_rhs = singles.tile([3, NPAIR * D], MMD, tag="big_rhs")

            def refree(sl: bass.AP, free_ap):
                # keep a slice's partition entry + offset, replace free dims
                return bass.AP(tensor=sl.tensor, offset=sl.offset,
                               ap=[list(sl.ap[0])] + free_ap)

            def rep_row(src_row, nrep):
                # repeat a [1, D] SBUF row nrep times along the free dim
                sl = src_row[0:1, 0:D]
                return refree(sl, [[0, nrep], [1, D]])

            def scr_bcast(idx):
                # scr_d[0, idx] (DRAM) replicated D times
                src = scr_d[0:1, idx:idx + 1]
                return bass.AP(tensor=src.tensor, offset=src.offset,
                               ap=[[0, 1], [0, D], [1, 1]])

            # scratch-dependent broadcasts on SP, in pair order
            for p in range(NPAIR):
                h = p % H
                cs = slice(p * D, (p + 1) * D)
                nc.sync.dma_start(out=big_lhsT[0:1, cs], in_=scr_bcast(h))
                nc.sync.dma_start(out=big_rhs[1:2, cs], in_=scr_bcast(H + h))
                nc.sync.dma_start(out=big_rhs[2:3, cs], in_=scr_bcast(2 * H + h))

            # x rows and ones rows: merged copies on the ACT queue
            for b in range(BPC):
                nc.scalar.dma_start(
                    out=big_lhsT[1:2, b * H * D:(b + 1) * H * D],
                    in_=rep_row(xrowR[b], H))
                nc.scalar.dma_start(
                    out=big_rhs[0:1, b * H * D:(b + 1) * H * D],
                    in_=rep_row(xrowR[b], H))
            nc.scalar.dma_start(out=big_lhsT[2:3, :], in_=rep_row(onesR, NPAIR))

            lhsT3 = [big_lhsT[0:3, p * D:(p + 1) * D] for p in range(NPAIR)]
            rhs3 = [big_rhs[0:3, p * D:(p + 1) * D] for p in range(NPAIR)]

            # ---------- main loops ----------
            # The value ops (px, ts-accum) and per-pair epilogues are
            # software-pipelined ACROSS pairs: the DVE/Pool streams never
            # drain at pair boundaries.
            acc_of = {}          # b -> running head accumulator tile
            pending_val = []     # [(b, h, t, p_t, ns8)]
            pending_epi = []     # [(b, h, z8, ns8)]

            def do_val(vb, vh, t, p_t, ns8):
                px_t = bigpx.tile([128, D], BF16, tag="px")
                if t in DVE_TT_TILES:
                    nc.vector.tensor_tensor(out=px_t, in0=p_t,
                                            in1=xbh[vb], op=OP.mult)
                else:
                    nc.gpsimd.tensor_tensor(out=px_t, in0=p_t,
                                            in1=x_bcast[vb], op=OP.mult)
                s_t = bigs.tile([128, D], BF16, tag="s")
                nc.vector.tensor_scalar(
                    out=s_t, in0=px_t, scalar1=avs[:, vh:vh + 1],
                    scalar2=0.0, op0=OP.mult, op1=OP.add,
                    accum_out=ns8[:, t:t + 1],
                )

            def do_epi(eb, eh, z8, ns8):
                # att_h = ns / Z ; acc += att_h; after the last head: y out
                rz8 = smalls.tile([128, NT], F32, tag="rz8")
                nc.vector.reciprocal(out=rz8, in_=z8)
                acc = acc_of.get(eb)
                acc_new = smalls.tile([128, NT], F32, tag=f"acc{eb}_{eh}")
                if acc is None:
                    nc.vector.tensor_tensor(out=acc_new, in0=ns8, in1=rz8,
                                            op=OP.mult)
                else:
                    t2 = smalls.tile([128, NT], F32, tag="t2")
                    nc.vector.tensor_tensor(out=t2, in0=ns8, in1=rz8,
                                            op=OP.mult)
                    nc.gpsimd.tensor_tensor(out=acc_new, in0=acc, in1=t2,
                                            op=OP.add)
                acc_of[eb] = acc_new
                if eh == H - 1:
                    yb8 = smalls.tile([128, NT], F32, tag="yb8")
                    nc.scalar.activation(out=yb8, in_=acc_new, func=AF.Identity,
                                         bias=bvsum, scale=1.0)
                    y8 = smalls.tile([128, NT], F32, tag="y8")
                    nc.vector.tensor_tensor(out=y8, in0=yb8, in1=x_col[eb],
                                            op=OP.add)
                    nc.sync.dma_start(out=y_col_v[eb], in_=y8)

            for b in range(BPC):
                for h in range(H):
                    p = b * H + h
                    lt = lhsT3[p]
                    rt = rhs3[p]

                    mt8 = smalls.tile([128, NT], F32, tag="mt8")
                    nm8 = smalls.tile([128, NT], F32, tag="nm8")
                    z8 = smalls.tile([128, NT], F32, tag="z8")
                    ns8 = smalls.tile([128, NT], F32, tag="ns8")

                    for t in range(NT):
                        d2 = psum.tile([128, D], F32, tag="d2")
                        lt_sl = lt[0:3, t * 128:(t + 1) * 128]
                        for c in range(2):
                            js = slice(c * 512, (c + 1) * 512)
                            nc.tensor.matmul(d2[:, js], lt_sl, rt[0:3, js],
                                             start=True, stop=True)
                        # fused: r = ~1/(|d|+eps) fp32, m = row max (fp32)
                        r_t = bigr.tile([128, D], F32, tag="r")
                        nc.vector._custom_dve(RECIP_OP, out=r_t, in0=d2,
                                              s0=C0V, s1=C1V, imm2=2.0 * EPS,
                                              accum_out=mt8[:, t:t + 1])
                        # bias = -m on ACT (slack engine)
                        nc.scalar.activation(out=nm8[:, t:t + 1],
                                             in_=mt8[:, t:t + 1],
                                             func=AF.Copy, scale=-1.0)
                        # p = exp(r - m), Z accum fp32; p dtype matches the
                        # tile's value-path engine (bf16 only where DVE's
                        # 2x tensor_tensor needs 2-byte operands)
                        p_t = bigp.tile([128, D],
                                        BF16 if t in DVE_TT_TILES else F32,
                                        tag="p")
                        nc.scalar.activation(out=p_t, in_=r_t, func=AF.Exp,
                                             bias=nm8[:, t:t + 1], scale=1.0,
                                             accum_out=z8[:, t:t + 1])
                        pending_val.append((b, h, t, p_t, ns8))
                        if len(pending_val) > 2:
                            do_val(*pending_val.pop(0))
                        # run the previous pair's epilogue early in this pair
                        if t == 2 and pending_epi:
                            do_epi(*pending_epi.pop(0))
                    pending_epi.append((b, h, z8, ns8))

            while pending_val:
                do_val(*pending_val.pop(0))
            while pending_epi:
                do_epi(*pending_epi.pop(0))

    nc.compile()
    return nc


_NC_CACHE = {}


def _get_nc():
    if "nc" not in _NC_CACHE:
        _NC_CACHE["nc"] = build_bass()
    return _NC_CACHE["nc"]


def kernel(**inputs) -> np.ndarray:
    x = np.ascontiguousarray(np.asarray(inputs["x"], dtype=np.float32))
    params = {
        k: np.ascontiguousarray(np.asarray(inputs[k], dtype=np.float32))
        for k in ("alpha_q", "beta_q", "alpha_k", "beta_k", "alpha_v", "beta_v")
    }
    nc = _get_nc()
    in_maps = []
    for c in range(N_CORES):
        m = {"x": x[c * BPC:(c + 1) * BPC]}
        m.update(params)
        in_maps.append(m)
    res = run_bass_kernel_spmd(nc, in_maps, core_ids=list(range(N_CORES)))
    return np.concatenate([r["y"] for r in res.results], axis=0)


if __name__ == "__main__":
    rng = np.random.default_rng(0)
    demo = {
        "x": rng.standard_normal((B, D), dtype=np.float32),
        "alpha_q": rng.random((1, H), dtype=np.float32),
        "beta_q": np.zeros((1, H), np.float32),
        "alpha_k": rng.random((1, H), dtype=np.float32),
        "beta_k": np.zeros((1, H), np.float32),
        "alpha_v": rng.random((1, H), dtype=np.float32),
        "beta_v": np.zeros((1, H), np.float32),
    }
    out = kernel(**demo)
    print("kernel output", out.shape, out.dtype)


# revision 22
# speedup vs baseline: 1.7217x; 1.0649x over previous
"""Trainium2 Bass kernel for nn_FLAttention (sparse_attention).

Math (per batch b, head h), with q = aq*x+bq, k = ak*x+bk, v = av*x+bv:
  S[i,j] = 1/(|k_j - q_i| + eps);  P = softmax_j(S);  att_i = sum_j P_ij v_j / sqrt(H)
  out = x + sum_h att

Pipeline per (b,h) pair and 128-query i-tile (D=1024 -> 8 tiles):
  PE  : dp[i,j] = (ak*x_j - aq*x_i + (bk-bq)+eps) via one K=3 matmul per
        512-chunk -> PSUM. Operand rows are built once in the prologue:
        lhsT = [ak*ones; x; ones], rhs = [x; -aq*ones; (bk-bq+eps)*ones],
        where the constant rows come from 0-stride broadcast DMAs (engine-free).
  DVE : custom fused op RECIP_ABSMAX_ANT reads dp from PSUM and computes
        r = recip1NR(max(dp, 2eps-dp)) = ~1/(|d|+eps) (0.4% approx) -> bf16,
        with a free row-max accumulate m (bf16, bit-consistent with r).
  ACT : p = Exp(r - m) -> bf16, accum_out Z (fp32). Row max of p is exactly 1.
  Pool/DVE (split): px = p * x_bcast (bf16 tensor_tensor)
  DVE : tensor_scalar(px * avs_h) at 4x bf16 with accum -> ns column.
  Epilogue per pair: att = ns * (1/Z); accumulate over heads; per batch:
        y = x + sum_h att + sum_h beta_v/sqrt(H).

The custom DVE op is registered at runtime (row 17 of the custom-DVE table);
its 8-stage body is: x=max(Src0, C2-Src0); ~x bit-flip seed; one Newton step.
Approximation error ~0.4% on r only perturbs softmax weight ties between keys
whose values are within the same distance scale - end-to-end rel err ~2e-4.

Sharding: data-parallel over batch: B=16 -> 2 batches per core on 8 cores.
"""
import numpy as np

import concourse.bass as bass
import concourse.bacc as bacc
import concourse.mybir as mybir
import concourse.tile as tile
from concourse.bass_utils import run_bass_kernel_spmd

B, D, H = 16, 1024, 4
N_CORES = 8
BPC = B // N_CORES          # batches per core
NPAIR = BPC * H             # (b,h) pairs per core
NT = D // 128               # i-tiles per pair
EPS = 1e-8
ISH = float(1.0 / np.sqrt(np.float32(H)))  # 1/sqrt(H) = 0.5

F32 = mybir.dt.float32
F32R = mybir.dt.float32r
BF16 = mybir.dt.bfloat16
AX = mybir.AxisListType
OP = mybir.AluOpType
AF = mybir.ActivationFunctionType

F32R_MM = True                      # fp32r matmuls (1 cyc/row vs 4)
DVE_TT_TILES = frozenset({3, 7})    # tiles whose p*x runs on DVE, rest on Pool

# ---------------- custom DVE op: r = ~1/(|d|+eps) with row-max accum --------
from concourse.dve_spec import (Spec, Src0, C0, C1, C2, Zero, Bin, AluOp,
                                 maxx, lower)
from concourse.dve_uop import DveOpSpec
from concourse.dve_ops import DveOp, RECIP_APPROX_FAST_CONSTS
import concourse.dve_ops as dve_ops

RECIP_NAME = "RECIP_ABSMAX_ANT"
C0V = RECIP_APPROX_FAST_CONSTS["s0"]
C1V = RECIP_APPROX_FAST_CONSTS["s1"]


def _recip_absmax_ref(in0, in1, c0, c1, c2):
    # in0 = d+eps; x = max(in0, c2-in0) = |d|+eps (c2 = 2eps);
    # out = 1-NR approx of 1/x; accum = max over free dim, seeded at 0
    x = np.maximum(in0.astype(np.float32),
                   (np.float32(c2) - in0).astype(np.float32))
    not_x = (~x.view(np.int32)).view(np.float32)
    y0 = not_x * np.float32(c0)
    y1 = (y0 * (np.float32(c1) - x * y0)).astype(np.float32)
    P = y1.shape[0]
    body = y1.reshape(P, -1)
    acc = np.maximum(np.float32(0.0), body.max(axis=-1, keepdims=True))
    return body, acc


def _register_recip_op():
    if RECIP_NAME in dve_ops._SUB_OPCODE_FOR_NAME:
        for o in dve_ops.OPS:
            if o.name == RECIP_NAME:
                return o
    x = Bin(AluOp.MAX, Src0, Bin(AluOp.SUBTRACT, C2, Src0))
    nx = Bin(AluOp.BITWISE_NOT, x, x)
    y0 = Bin(AluOp.MULTIPLY, nx, C0)
    t = Bin(AluOp.MULTIPLY, x, y0)
    y1 = Bin(AluOp.MULTIPLY, y0, Bin(AluOp.SUBTRACT, C1, t))
    spec = Spec(body=y1, accum=maxx, accum_init=Zero, reference=_recip_absmax_ref)
    row = max(dve_ops._SUB_OPCODE_FOR_NAME.values()) + 1
    assert row < 0x20
    dve_ops._SUB_OPCODE_FOR_NAME[RECIP_NAME] = row
    shas = {}
    for ver in ("v3", "v4"):
        s = DveOpSpec(name=RECIP_NAME, opcode=row, uops=lower(spec, ver=ver),
                      rd1_en=False)
        shas[ver] = s.sha(ver)
    op = DveOp(RECIP_NAME, spec, subdim=False, uops_sha=shas)
    dve_ops.OPS.append(op)
    dve_ops.CUSTOM_DVE_SPECS[RECIP_NAME] = spec
    return op


RECIP_OP = _register_recip_op()


def build_bass():
    nc = bacc.Bacc(
        "TRN2",
        target_bir_lowering=False,
        debug=False,
        enable_asserts=False,
        num_devices=N_CORES,
    )
    x_d = nc.dram_tensor("x", (BPC, D), F32, kind="ExternalInput").ap()
    aq_d = nc.dram_tensor("alpha_q", (1, H), F32, kind="ExternalInput").ap()
    bq_d = nc.dram_tensor("beta_q", (1, H), F32, kind="ExternalInput").ap()
    ak_d = nc.dram_tensor("alpha_k", (1, H), F32, kind="ExternalInput").ap()
    bk_d = nc.dram_tensor("beta_k", (1, H), F32, kind="ExternalInput").ap()
    av_d = nc.dram_tensor("alpha_v", (1, H), F32, kind="ExternalInput").ap()
    bv_d = nc.dram_tensor("beta_v", (1, H), F32, kind="ExternalInput").ap()
    y_d = nc.dram_tensor("y", (BPC, D), F32, kind="ExternalOutput").ap()

    x_col_v = x_d.rearrange("b (t p) -> b p t", p=128)
    y_col_v = y_d.rearrange("b (t p) -> b p t", p=128)

    MMD = F32R if F32R_MM else F32
    # DRAM scratch for matmul const-row sources: [ak(H); -aq(H); (bk-bq)+eps(H)]
    scr_d = nc.dram_tensor("const_scratch", (1, 3 * H), MMD, kind="Internal").ap()

    def bcast_part(src: bass.AP, n_part: int):
        # replicate a (1, n) row across n_part partitions (0-stride DMA)
        return bass.AP(tensor=src.tensor, offset=src.offset,
                       ap=[[0, n_part]] + list(src.ap[1:]))

    def bcast_free(src_ap: bass.AP, n: int):
        # replicate a single DRAM element along the free dim (0-stride mid dim)
        return bass.AP(tensor=src_ap.tensor, offset=src_ap.offset,
                       ap=[[0, 1], [0, n], [1, 1]])

    with tile.TileContext(nc) as tc:
        with (
            tc.tile_pool(name="singles", bufs=1) as singles,
            tc.tile_pool(name="psum", bufs=3, space="PSUM") as psum,
            tc.tile_pool(name="bigr", bufs=5) as bigr,       # r tiles
            tc.tile_pool(name="bigp", bufs=7) as bigp,       # p tiles
            tc.tile_pool(name="bigpx", bufs=5) as bigpx,     # px tiles (bf16)
            tc.tile_pool(name="bigs", bufs=3) as bigs,       # ts-val discard
            tc.tile_pool(name="smalls", bufs=6) as smalls,
        ):
            # ---------- one-time prep ----------
            # params first on the SP queue (head of the const-row chain)
            def param_row(src, nm):
                t = singles.tile([1, H], F32, tag=nm)
                nc.sync.dma_start(out=t, in_=src)
                return t

            aqP = param_row(aq_d, "aqP")
            akP = param_row(ak_d, "akP")
            bqP = param_row(bq_d, "bqP")
            bkP = param_row(bk_d, "bkP")

            # x assets on the ACT queue (parallel with SP's stream)
            x_bcast = []
            xbh = []
            x_col = []
            xrow = []
            for b in range(BPC):
                xb = singles.tile([128, D], F32, tag=f"x_bcast{b}")
                nc.scalar.dma_start(
                    out=xb,
                    in_=bass.AP(tensor=x_d.tensor, offset=x_d.offset + b * D,
                                ap=[[0, 128], [1, D]]),
                )
                x_bcast.append(xb)
                xh = singles.tile([128, D], BF16, tag=f"xbh{b}")
                nc.gpsimd.tensor_copy(out=xh, in_=xb)
                xbh.append(xh)
                xc = singles.tile([128, NT], F32, tag=f"x_col{b}")
                nc.scalar.dma_start(out=xc, in_=x_col_v[b])
                x_col.append(xc)
                xr = singles.tile([1, D], F32, tag=f"xrow{b}")
                nc.scalar.dma_start(out=xr, in_=x_d[b:b + 1, :])
                xrow.append(xr)

            naqP = singles.tile([1, H], F32, tag="naqP")   # -alpha_q
            nc.vector.tensor_scalar(out=naqP, in0=aqP, scalar1=-1.0, scalar2=None,
                                    op0=OP.mult)
            ccP = singles.tile([1, H], F32, tag="ccP")     # beta_k - beta_q
            nc.vector.tensor_tensor(out=ccP, in0=bkP, in1=bqP, op=OP.subtract)
            cceP = singles.tile([1, H], F32, tag="cceP")   # (bk-bq) + eps
            nc.vector.tensor_scalar(out=cceP, in0=ccP, scalar1=1.0, scalar2=EPS,
                                    op0=OP.mult, op1=OP.add)

            ones_row = singles.tile([1, D], F32, tag="ones_row")
            nc.gpsimd.memset(ones_row, 1.0)

            # const-row sources: [ak; -aq; cce] rounded to MMD, staged in DRAM
            consts3 = singles.tile([1, 3 * H], MMD, tag="consts3")
            nc.vector.tensor_copy(out=consts3[0:1, 0:H], in_=akP)
            nc.vector.tensor_copy(out=consts3[0:1, H:2 * H], in_=naqP)
            nc.vector.tensor_copy(out=consts3[0:1, 2 * H:3 * H], in_=cceP)
            nc.sync.dma_start(out=scr_d, in_=consts3)

            onesR = singles.tile([1, D], MMD, tag="onesR")
            if F32R_MM:
                nc.gpsimd.tensor_copy(out=onesR, in_=ones_row)
                xrowR = []
                for b in range(BPC):
                    xrr = singles.tile([1, D], F32R, tag=f"xrowR{b}")
                    nc.gpsimd.tensor_copy(out=xrr, in_=xrow[b])
                    xrowR.append(xrr)
            else:
                nc.sync.dma_start(out=onesR, in_=ones_row)
                xrowR = xrow

            # value-path params on the ACT queue
            av128 = singles.tile([128, H], F32, tag="av128")
            nc.scalar.dma_start(out=av128, in_=bcast_part(av_d, 128))
            avs = singles.tile([128, H], F32, tag="avs")
            nc.vector.tensor_scalar(out=avs, in0=av128, scalar1=ISH, scalar2=None,
                                    op0=OP.mult)
            bv128 = singles.tile([128, H], F32, tag="bv128")
            nc.scalar.dma_start(out=bv128, in_=bcast_part(bv_d, 128))
            bvs = singles.tile([128, H], F32, tag="bvs")
            nc.vector.tensor_scalar(out=bvs, in0=bv128, scalar1=ISH, scalar2=None,
                                    op0=OP.mult)
            bvsum = singles.tile([128, 1], F32, tag="bvsum")
            nc.vector.tensor_reduce(out=bvsum, in_=bvs, axis=AX.X, op=OP.add)

            # persistent matmul operands, one big tile per side; pair p's
            # operand block is columns [p*D, (p+1)*D):
            #   lhsT rows = [ak; x; 1], rhs rows = [x; -aq; cce]
            big_lhsT = singles.tile([3, NPAIR * D], MMD, tag="big_lhsT")
            big_rhs = singles.tile([3, NPAIR * D], MMD, tag="big_rhs")

            def refree(sl: bass.AP, free_ap):
                # keep a slice's partition entry + offset, replace free dims
                return bass.AP(tensor=sl.tensor, offset=sl.offset,
                               ap=[list(sl.ap[0])] + free_ap)

            def rep_row(src_row, nrep):
                # repeat a [1, D] SBUF row nrep times along the free dim
                sl = src_row[0:1, 0:D]
                return refree(sl, [[0, nrep], [1, D]])

            def scr_bcast(idx, n):
                # scr_d[0, idx] (DRAM) replicated n times
                src = scr_d[0:1, idx:idx + 1]
                return bass.AP(tensor=src.tensor, offset=src.offset,
                               ap=[[0, 1], [0, n], [1, 1]])

            # column layout is h-major (q = h*BPC + b) so each head's const
            # broadcast covers BPC adjacent chunks in one DMA
            for h in range(H):
                cs = slice(h * BPC * D, (h + 1) * BPC * D)
                nc.sync.dma_start(out=big_lhsT[0:1, cs],
                                  in_=scr_bcast(h, BPC * D))
                nc.sync.dma_start(out=big_rhs[1:2, cs],
                                  in_=scr_bcast(H + h, BPC * D))
                nc.sync.dma_start(out=big_rhs[2:3, cs],
                                  in_=scr_bcast(2 * H + h, BPC * D))

            # x rows: one strided merged copy per batch per side (ACT queue);
            # ones row: one contiguous copy
            for b in range(BPC):
                nc.scalar.dma_start(
                    out=refree(big_lhsT[1:2, b * D:b * D + 1],
                               [[BPC * D, H], [1, D]]),
                    in_=rep_row(xrowR[b], H))
                nc.scalar.dma_start(
                    out=refree(big_rhs[0:1, b * D:b * D + 1],
                               [[BPC * D, H], [1, D]]),
                    in_=rep_row(xrowR[b], H))
            nc.scalar.dma_start(out=big_lhsT[2:3, :], in_=rep_row(onesR, NPAIR))

            def opcol(p):
                b, h = divmod(p, H)
                q = h * BPC + b
                return slice(q * D, (q + 1) * D)

            lhsT3 = [big_lhsT[0:3, opcol(p)] for p in range(NPAIR)]
            rhs3 = [big_rhs[0:3, opcol(p)] for p in range(NPAIR)]

            # ---------- main loops ----------
            # The value ops (px, ts-accum) and per-pair epilogues are
            # software-pipelined ACROSS pairs: the DVE/Pool streams never
            # drain at pair boundaries.
            acc_of = {}          # b -> running head accumulator tile
            pending_val = []     # [(b, h, t, p_t, ns8)]
            pending_epi = []     # [(b, h, z8, ns8)]

            def do_val(vb, vh, t, p_t, ns8):
                px_t = bigpx.tile([128, D], BF16, tag="px")
                if t in DVE_TT_TILES:
                    nc.vector.tensor_tensor(out=px_t, in0=p_t,
                                            in1=xbh[vb], op=OP.mult)
                else:
                    nc.gpsimd.tensor_tensor(out=px_t, in0=p_t,
                                            in1=x_bcast[vb], op=OP.mult)
                s_t = bigs.tile([128, D], BF16, tag="s")
                nc.vector.tensor_scalar(
                    out=s_t, in0=px_t, scalar1=avs[:, vh:vh + 1],
                    scalar2=0.0, op0=OP.mult, op1=OP.add,
                    accum_out=ns8[:, t:t + 1],
                )

            def do_epi(eb, eh, z8, ns8):
                # att_h = ns / Z ; acc += att_h; after the last head: y out
                rz8 = smalls.tile([128, NT], F32, tag="rz8")
                nc.vector.reciprocal(out=rz8, in_=z8)
                acc = acc_of.get(eb)
                acc_new = smalls.tile([128, NT], F32, tag=f"acc{eb}_{eh}")
                if acc is None:
                    nc.vector.tensor_tensor(out=acc_new, in0=ns8, in1=rz8,
                                            op=OP.mult)
                else:
                    t2 = smalls.tile([128, NT], F32, tag="t2")
                    nc.vector.tensor_tensor(out=t2, in0=ns8, in1=rz8,
                                            op=OP.mult)
                    nc.gpsimd.tensor_tensor(out=acc_new, in0=acc, in1=t2,
                                            op=OP.add)
                acc_of[eb] = acc_new
                if eh == H - 1:
                    yb8 = smalls.tile([128, NT], F32, tag="yb8")
                    nc.scalar.activation(out=yb8, in_=acc_new, func=AF.Identity,
                                         bias=bvsum, scale=1.0)
                    y8 = smalls.tile([128, NT], F32, tag="y8")
                    nc.vector.tensor_tensor(out=y8, in0=yb8, in1=x_col[eb],
                                            op=OP.add)
                    nc.sync.dma_start(out=y_col_v[eb], in_=y8)

            for b in range(BPC):
                for h in range(H):
                    p = b * H + h
                    lt = lhsT3[p]
                    rt = rhs3[p]

                    mt8 = smalls.tile([128, NT], F32, tag="mt8")
                    nm8 = smalls.tile([128, NT], F32, tag="nm8")
                    z8 = smalls.tile([128, NT], F32, tag="z8")
                    ns8 = smalls.tile([128, NT], F32, tag="ns8")

                    for t in range(NT):
                        d2 = psum.tile([128, D], F32, tag="d2")
                        lt_sl = lt[0:3, t * 128:(t + 1) * 128]
                        for c in range(2):
                            js = slice(c * 512, (c + 1) * 512)
                            nc.tensor.matmul(d2[:, js], lt_sl, rt[0:3, js],
                                             start=True, stop=True)
                        # fused: r = ~1/(|d|+eps) fp32, m = row max (fp32)
                        r_t = bigr.tile([128, D], F32, tag="r")
                        nc.vector._custom_dve(RECIP_OP, out=r_t, in0=d2,
                                              s0=C0V, s1=C1V, imm2=2.0 * EPS,
                                              accum_out=mt8[:, t:t + 1])
                        # bias = -m on ACT (slack engine)
                        nc.scalar.activation(out=nm8[:, t:t + 1],
                                             in_=mt8[:, t:t + 1],
                                             func=AF.Copy, scale=-1.0)
                        # p = exp(r - m), Z accum fp32; p dtype matches the
                        # tile's value-path engine (bf16 only where DVE's
                        # 2x tensor_tensor needs 2-byte operands)
                        p_t = bigp.tile([128, D],
                                        BF16 if t in DVE_TT_TILES else F32,
                                        tag="p")
                        nc.scalar.activation(out=p_t, in_=r_t, func=AF.Exp,
                                             bias=nm8[:, t:t + 1], scale=1.0,
                                             accum_out=z8[:, t:t + 1])
                        pending_val.append((b, h, t, p_t, ns8))
                        if len(pending_val) > 3:
                            do_val(*pending_val.pop(0))
                        # run the previous pair's epilogue early in this pair
                        if t == 2 and pending_epi:
                            do_epi(*pending_epi.pop(0))
                    pending_epi.append((b, h, z8, ns8))

            while pending_val:
                do_val(*pending_val.pop(0))
            while pending_epi:
                do_epi(*pending_epi.pop(0))

    nc.compile()
    return nc


_NC_CACHE = {}


def _get_nc():
    if "nc" not in _NC_CACHE:
        _NC_CACHE["nc"] = build_bass()
    return _NC_CACHE["nc"]


def kernel(**inputs) -> np.ndarray:
    x = np.ascontiguousarray(np.asarray(inputs["x"], dtype=np.float32))
    params = {
        k: np.ascontiguousarray(np.asarray(inputs[k], dtype=np.float32))
        for k in ("alpha_q", "beta_q", "alpha_k", "beta_k", "alpha_v", "beta_v")
    }
    nc = _get_nc()
    in_maps = []
    for c in range(N_CORES):
        m = {"x": x[c * BPC:(c + 1) * BPC]}
        m.update(params)
        in_maps.append(m)
    res = run_bass_kernel_spmd(nc, in_maps, core_ids=list(range(N_CORES)))
    return np.concatenate([r["y"] for r in res.results], axis=0)


if __name__ == "__main__":
    rng = np.random.default_rng(0)
    demo = {
        "x": rng.standard_normal((B, D), dtype=np.float32),
        "alpha_q": rng.random((1, H), dtype=np.float32),
        "beta_q": np.zeros((1, H), np.float32),
        "alpha_k": rng.random((1, H), dtype=np.float32),
        "beta_k": np.zeros((1, H), np.float32),
        "alpha_v": rng.random((1, H), dtype=np.float32),
        "beta_v": np.zeros((1, H), np.float32),
    }
    out = kernel(**demo)
    print("kernel output", out.shape, out.dtype)
